# revision 2
# baseline (speedup 1.0000x reference)
"""NNUE forward kernel for Trainium2, 8-core SPMD, batch-sharded,
sparsity-exploiting (embedding-gather formulation), fp8 tables +
identity-matrix reduction.

Reference computation (B=4096, I=40960, H=256):
    h_p = clip(x_p @ W_p.T + b_p, 0, 1)   for p in {1,2}
    out = concat(h1, h2) @ v + b2         -> (B,)

x_p rows are sparse binary (~30 active features of 40960), so
x_p @ W_p.T is an embedding-sum: h[b] = sum_{active f} W_p.T[f, :].

Tables are fp8 e4m3, pre-scaled by 2^15 so values sit in e4m3's normal
range (weights are ~U(-1/202, 1/202)); the scale folds into the
epilogue: h = clip(psum, 0, SCALE) * (v / SCALE). Measured end-to-end
norm-rel error of e4m3 quantization on this data: 5.8e-3 (tolerance
2e-2).

Identity-matrix reduction: the host assigns each batch row a fixed
budget of B_ID=16 gather lanes per table half: the j-th gathered slot
of a block lands on SBUF partition j%128, and we place row r's
features on lane r. The PE reduction over each pair of 128-slot blocks
is then lhsT = a constant [128,2,128] identity (DoubleRow fp8 matmul,
2x rate), so no per-iteration selector-matrix upload is needed. Rows
with more than 16 features per half spill to a small overflow gather
(<=2 blocks of 128) reduced with a tiny uploaded one-hot S. The l1
bias enters the PSUM group as a rank-1 matmul (ones[1,128] x
b1s[1,256]). Unused identity lanes gather a zero table row; overflow
padding uses trailing -1 indices (skipped by the SWDGE ucode) with an
exact per-(iter,half) count register, except the first BUFS iterations
whose fresh SBUF buffers must be fully overwritten to avoid NaN
garbage under the S=0 columns.

Per core (512 batch rows): 8 iterations over (128-row tile,
perspective); per iteration 6 dma_gathers (2x 1024-idx per half +
2x 256-idx overflow; <=1024 idxs per call -- the SWDGE ucode wedges
beyond that; 4 SWDGE queues round-robin), 8 identity DoubleRow
matmuls + 2 overflow DoubleRow + 1 bias matmul into PSUM, then a
Vector epilogue (relu off PSUM, fused min/dot with v/SCALE, + b2).
No collectives (pure data parallel; batch-sharded).
"""

import numpy as np
import ml_dtypes

import concourse.bass as bass
import concourse.mybir as mybir
from concourse import bacc
from concourse.tile import TileContext
from concourse.bass_utils import run_bass_kernel_spmd

BATCH = 4096
INPUT_SIZE = 40960
HIDDEN = 256
N_CORES = 8
B_CORE = BATCH // N_CORES  # 512
N_TILES = B_CORE // 128  # 4
HALF = INPUT_SIZE // 2  # 20480 rows per table half (int16 index range)
ZR = HALF  # zero row index (padding target)

SCALE = 2.0**15  # fp8 pre-scale; folded into epilogue clip + v
B_ID = 16  # identity-lane budget per row per half
OVFN = 256  # overflow slots per (tile, persp, half): 2 blocks of 128
BUFS = 3  # gather pool depth; first BUFS iterations fully pad
N_ITER = 2 * N_TILES  # 8 (tile, persp) iterations
IDX_COLS_ITER = 4 * 64 + 2 * (OVFN // 16)  # idx cols per iter (int16 16-wrap)

BF16 = mybir.dt.bfloat16
F32 = mybir.dt.float32
F8 = mybir.dt.float8e4
I16 = mybir.dt.int16

NP_F8 = ml_dtypes.float8_e4m3

_NC_CACHE = {}


def _build():
    nc = bacc.Bacc(
        "TRN2", target_bir_lowering=False, debug=False, num_swdge_queues=4
    )

    tbl = [
        [
            nc.dram_tensor(f"t{p}{h}", [HALF + 1, HIDDEN], F8, kind="ExternalInput")
            for h in range(2)
        ]
        for p in range(2)
    ]
    idxd = nc.dram_tensor(
        "idx", [128, N_ITER * IDX_COLS_ITER], I16, kind="ExternalInput"
    )
    cntd = nc.dram_tensor("cnt", [1, 16], mybir.dt.uint32, kind="ExternalInput")
    sovfd = nc.dram_tensor(
        "sovf", [N_ITER, 128, 2 * OVFN], F8, kind="ExternalInput"
    )
    i2d = nc.dram_tensor("i2", [128, 2 * 128], F8, kind="ExternalInput")
    onesd = nc.dram_tensor("ones", [1, 128], F8, kind="ExternalInput")
    b1sd = nc.dram_tensor("b1s", [1, 2 * HIDDEN], F8, kind="ExternalInput")
    vd = nc.dram_tensor("v", [128, 2, HIDDEN], F32, kind="ExternalInput")
    b2d = nc.dram_tensor("b2", [128, 1], F32, kind="ExternalInput")
    outd = nc.dram_tensor("out", [128, N_TILES], F32, kind="ExternalOutput")

    with TileContext(nc) as tc:
        with (
            tc.tile_pool(name="consts", bufs=1) as consts,
            tc.tile_pool(name="gp", bufs=BUFS) as gp,
            tc.tile_pool(name="psum", bufs=6, space="PSUM") as pp,
            tc.tile_pool(name="ep", bufs=2) as ep,
        ):
            idxt = consts.tile([128, N_ITER, IDX_COLS_ITER], I16, tag="idx")
            nc.sync.dma_start(out=idxt[:, :, :], in_=idxd[:, :])
            # Exact overflow counts per (iter, half); trailing -1 idxs are
            # skipped by the ucode (no descriptors, no DMA); the count
            # register must match exactly.
            cnt_t = consts.tile([1, 16], mybir.dt.uint32, tag="cnt")
            nc.sync.dma_start(out=cnt_t, in_=cntd[:, :])
            cregs = []
            for g in range(16):
                r = nc.gpsimd.alloc_register(f"cnt{g}")
                nc.gpsimd.reg_load(r, cnt_t[0:1, g : g + 1])
                cregs.append(r)
            i2_t = consts.tile([128, 2, 128], F8, tag="i2")
            nc.sync.dma_start(out=i2_t, in_=i2d[:, :])
            ones_t = consts.tile([1, 128], F8, tag="ones")
            nc.sync.dma_start(out=ones_t, in_=onesd[:, :])
            b1s_t = consts.tile([1, 2, HIDDEN], F8, tag="b1s")
            nc.sync.dma_start(out=b1s_t, in_=b1sd[:, :])
            v_t = consts.tile([128, 2, HIDDEN], F32, tag="v")
            nc.sync.dma_start(out=v_t, in_=vd[:, :, :])
            b2_t = consts.tile([128, 1], F32, tag="b2")
            nc.sync.dma_start(out=b2_t, in_=b2d[:, :])
            outst = consts.tile([128, N_TILES], F32, tag="outst")
            sovf_t = consts.tile([128, N_ITER, 2, 2, 128], F8, tag="sovf")
            for i in range(N_ITER):
                nc.sync.dma_start(out=sovf_t[:, i, :, :, :], in_=sovfd[i, :, :])

            qn = 0
            acc0 = None
            for i in range(N_ITER):
                t, p = i // 2, i % 2
                ib = i * IDX_COLS_ITER
                # Gathers: per half 2 chunks of 1024 (8 blocks each, always
                # full: identity lanes pad with the zero row), then per half
                # one 256-idx overflow chunk with exact count.
                gts = []
                for h in range(2):
                    for cch in range(2):
                        gt = gp.tile([128, 8, HIDDEN], F8, tag=f"g{h}{cch}")
                        c0 = ib + (2 * h + cch) * 64
                        nc.gpsimd.dma_gather(
                            gt,
                            tbl[p][h][:, :],
                            idxt[:, i, c0 - ib : c0 - ib + 64],
                            1024,
                            1024,
                            HIDDEN,
                            queue_num=qn % 4,
                        )
                        qn += 1
                        gts.append(gt)
                ovf = []
                for h in range(2):
                    ot = gp.tile([128, OVFN // 128, HIDDEN], F8, tag=f"o{h}")
                    c0 = 4 * 64 + h * (OVFN // 16)
                    nreg = OVFN if i < BUFS else cregs[2 * i + h]
                    nc.gpsimd.dma_gather(
                        ot,
                        tbl[p][h][:, :],
                        idxt[:, i, c0 : c0 + OVFN // 16],
                        OVFN,
                        nreg,
                        HIDDEN,
                        queue_num=qn % 4,
                    )
                    qn += 1
                    ovf.append(ot)

                psum = pp.tile([128, HIDDEN], F32, tag="psum")
                # l1 bias as a rank-1 matmul: ones[1,128].T @ b1s[1,256].
                nc.tensor.matmul(
                    psum,
                    lhsT=ones_t[0:1, :],
                    rhs=b1s_t[0:1, p, :],
                    start=True,
                    stop=False,
                )
                # Identity DoubleRow matmuls: psum[r,:] += G[r,2c,:]+G[r,2c+1,:]
                for gt in gts:
                    for c2 in range(4):
                        nc.tensor.matmul(
                            psum,
                            lhsT=i2_t[:, :, :],
                            rhs=gt[:, 2 * c2 : 2 * c2 + 2, :],
                            perf_mode=mybir.MatmulPerfMode.DoubleRow,
                            start=False,
                            stop=False,
                        )
                # Overflow: small one-hot S per (iter, half).
                for h in range(2):
                    nc.tensor.matmul(
                        psum,
                        lhsT=sovf_t[:, i, h, :, :],
                        rhs=ovf[h][:, :, :],
                        perf_mode=mybir.MatmulPerfMode.DoubleRow,
                        start=False,
                        stop=(h == 1),
                    )

                # Epilogue: relu off PSUM, fused (min SCALE, * v/SCALE),
                # reduce; combine perspectives + b2.
                clr = ep.tile([128, HIDDEN], F32, tag="clr")
                nc.vector.tensor_scalar_max(clr, psum, 0.0)
                prod = ep.tile([128, HIDDEN], F32, tag="prod")
                nc.vector.scalar_tensor_tensor(
                    prod,
                    clr,
                    SCALE,
                    v_t[:, p, :],
                    op0=mybir.AluOpType.min,
                    op1=mybir.AluOpType.mult,
                )
                if p == 0:
                    acc0 = ep.tile([128, 1], F32, tag="acc0")
                    nc.vector.tensor_reduce(
                        acc0, prod, axis=mybir.AxisListType.X, op=mybir.AluOpType.add
                    )
                else:
                    acc1 = ep.tile([128, 1], F32, tag="acc1")
                    nc.vector.tensor_reduce(
                        acc1, prod, axis=mybir.AxisListType.X, op=mybir.AluOpType.add
                    )
                    # out[:, t] = (acc0 + b2) + acc1
                    nc.vector.scalar_tensor_tensor(
                        outst[:, t : t + 1],
                        acc0,
                        b2_t,
                        acc1,
                        op0=mybir.AluOpType.add,
                        op1=mybir.AluOpType.add,
                    )
            nc.sync.dma_start(out=outd[:, :], in_=outst)

    nc.compile()
    return nc


def _wrap16(v):
    """Linear idx vector -> [16, n/16] SWDGE wrap, tiled to 128 partitions."""
    n = len(v)
    return np.tile(v.reshape(n // 16, 16).T, (8, 1))  # [128, n//16]


def _prep(x1, x2, l1_weights, l1_biases, l2_weight, l2_bias):
    """Host-side: fp8 tables, per-core identity-lane index lists, overflow
    S matrices, epilogue constants."""
    wt = l1_weights.astype(np.float32).transpose(0, 2, 1)  # [2, I, H]
    tabs = {}
    for p in range(2):
        for h in range(2):
            tt = np.zeros((HALF + 1, HIDDEN), dtype=NP_F8)
            tt[:HALF] = (wt[p, h * HALF : (h + 1) * HALF] * SCALE).astype(NP_F8)
            tabs[f"t{p}{h}"] = tt

    i2 = np.zeros((128, 2, 128), NP_F8)
    for tcol in range(2):
        i2[np.arange(128), tcol, np.arange(128)] = 1.0
    ones = np.ones((1, 128), NP_F8)
    b1s = (l1_biases.astype(np.float32).reshape(1, 2 * HIDDEN) * SCALE).astype(
        NP_F8
    )
    v_full = np.ascontiguousarray(
        np.broadcast_to(
            (l2_weight.astype(np.float32) / SCALE).reshape(1, 2, HIDDEN),
            (128, 2, HIDDEN),
        )
    )
    b2_full = np.full((128, 1), float(np.asarray(l2_bias).reshape(-1)[0]), np.float32)

    xs = [np.asarray(x1), np.asarray(x2)]
    in_maps = []
    for c in range(N_CORES):
        idx_arr = np.empty((N_ITER, 128, IDX_COLS_ITER), np.int16)
        sovf = np.zeros((N_ITER, 128, 2, 2, 128), NP_F8)
        cnt = np.zeros((1, 16), np.uint32)
        for i in range(N_ITER):
            t, p = i // 2, i % 2
            blk = xs[p][c * B_CORE + t * 128 : c * B_CORE + (t + 1) * 128]
            r_all, f_all = np.nonzero(blk)
            cols = []
            for h in range(2):
                sel = (f_all >= h * HALF) & (f_all < (h + 1) * HALF)
                r, f = r_all[sel], f_all[sel] - h * HALF
                # identity lanes: row r's first B_ID features at blocks 0..,
                # lane r; rest to overflow
                blk_idx = np.full((2 * 8, 128), ZR, np.int16)
                pos = np.zeros(128, np.int64)
                ovf_f, ovf_r = [], []
                for rr, ff in zip(r, f):
                    if pos[rr] < B_ID:
                        blk_idx[pos[rr], rr] = ff
                        pos[rr] += 1
                    else:
                        ovf_f.append(ff)
                        ovf_r.append(rr)
                cols.append(_wrap16(blk_idx[:8].reshape(1024)))
                cols.append(_wrap16(blk_idx[8:].reshape(1024)))
                m = len(ovf_f)
                assert m <= OVFN, (c, i, h, m)
                if i < BUFS:
                    ov = np.full(OVFN, ZR, np.int16)
                    cnt[0, 2 * i + h] = OVFN
                else:
                    ov = np.full(OVFN, -1, np.int16)
                    cnt[0, 2 * i + h] = max(m, 1)
                    if m == 0:
                        ov[0] = ZR
                ov[:m] = ovf_f
                j = np.arange(m)
                sovf[i, j % 128, h, j // 128, ovf_r] = 1.0
                cols.append(_wrap16(ov))
            # order: h0c0, h0c1, h1c0, h1c1, ovf0, ovf1
            idx_arr[i] = np.concatenate(
                [cols[0], cols[1], cols[3], cols[4], cols[2], cols[5]], axis=1
            )
        in_map = dict(tabs)
        in_map.update(
            idx=np.ascontiguousarray(
                idx_arr.transpose(1, 0, 2).reshape(128, N_ITER * IDX_COLS_ITER)
            ),
            cnt=cnt,
            sovf=np.ascontiguousarray(sovf.reshape(N_ITER, 128, 2 * OVFN)),
            i2=np.ascontiguousarray(i2.reshape(128, 2 * 128)),
            ones=ones,
            b1s=b1s,
            v=v_full,
            b2=b2_full,
        )
        in_maps.append(in_map)
    return in_maps


def _run(x1, x2, l1_weights, l1_biases, l2_weight, l2_bias, trace=False):
    in_maps = _prep(x1, x2, l1_weights, l1_biases, l2_weight, l2_bias)
    if "nc" not in _NC_CACHE:
        _NC_CACHE["nc"] = _build()
    nc = _NC_CACHE["nc"]

    res = run_bass_kernel_spmd(
        nc, in_maps, core_ids=list(range(N_CORES)), trace=trace
    )
    out = np.concatenate(
        [
            np.ascontiguousarray(res.results[c]["out"].T).reshape(B_CORE)
            for c in range(N_CORES)
        ]
    )
    return out.astype(np.float32), res


def kernel(**inputs):
    out, _ = _run(**inputs)
    return out


def kernel_profiled(**inputs):
    _, res = _run(**inputs, trace=True)
    return res


# revision 5
# speedup vs baseline: 1.0374x; 1.0374x over previous
"""NNUE forward kernel for Trainium2, 8-core SPMD, batch-sharded,
sparsity-exploiting (embedding-gather formulation), fp8 tables +
identity-matrix reduction.

Reference computation (B=4096, I=40960, H=256):
    h_p = clip(x_p @ W_p.T + b_p, 0, 1)   for p in {1,2}
    out = concat(h1, h2) @ v + b2         -> (B,)

x_p rows are sparse binary (~30 active features of 40960), so
x_p @ W_p.T is an embedding-sum: h[b] = sum_{active f} W_p.T[f, :].

Tables are fp8 e4m3, pre-scaled by 2^15 so values sit in e4m3's normal
range (weights are ~U(-1/202, 1/202)); the scale folds into the
epilogue: h = clip(psum, 0, SCALE) * (v / SCALE). Measured end-to-end
norm-rel error of e4m3 quantization on this data: 5.8e-3 (tolerance
2e-2).

Identity-matrix reduction: the host assigns each batch row a fixed
budget of B_ID=16 gather lanes per table half: the j-th gathered slot
of a block lands on SBUF partition j%128, and we place row r's
features on lane r. The PE reduction over each pair of 128-slot blocks
is then lhsT = a constant [128,2,128] identity (DoubleRow fp8 matmul,
2x rate), so no per-iteration selector-matrix upload is needed. Rows
with more than 16 features per half spill to a small overflow gather
(<=2 blocks of 128) reduced with a tiny uploaded one-hot S. The l1
bias enters the PSUM group as a rank-1 matmul (ones[1,128] x
b1s[1,256]). Unused identity lanes gather a zero table row; overflow
padding uses trailing -1 indices (skipped by the SWDGE ucode) with an
exact per-(iter,half) count register, except the first BUFS iterations
whose fresh SBUF buffers must be fully overwritten to avoid NaN
garbage under the S=0 columns.

Per core (512 batch rows): 8 iterations over (128-row tile,
perspective); per iteration 6 dma_gathers (2x 1024-idx per half +
2x 256-idx overflow; <=1024 idxs per call -- the SWDGE ucode wedges
beyond that; 4 SWDGE queues round-robin), 8 identity DoubleRow
matmuls + 2 overflow DoubleRow + 1 bias matmul into PSUM, then a
Vector epilogue (relu off PSUM, fused min/dot with v/SCALE, + b2).
No collectives (pure data parallel; batch-sharded).
"""

import numpy as np
import ml_dtypes

import concourse.bass as bass
import concourse.mybir as mybir
from concourse import bacc
from concourse.tile import TileContext
from concourse.bass_utils import run_bass_kernel_spmd

BATCH = 4096
INPUT_SIZE = 40960
HIDDEN = 256
N_CORES = 8
B_CORE = BATCH // N_CORES  # 512
N_TILES = B_CORE // 128  # 4
HALF = INPUT_SIZE // 2  # 20480 rows per table half (int16 index range)
ZR = HALF  # zero row index (padding target)

SCALE = 2.0**15  # fp8 pre-scale; folded into epilogue clip + v
B_ID = 16  # identity-lane budget per row per half
OVFN = 256  # overflow slots per (tile, persp, half): 2 blocks of 128
BUFS = 4  # gather pool depth; first BUFS iterations fully pad
SINGLE_PACKET = False  # let the SWDGE ucode packetize 256B descriptors
N_ITER = 2 * N_TILES  # 8 (tile, persp) iterations
IDX_COLS_ITER = 4 * 64 + 2 * (OVFN // 16)  # idx cols per iter (int16 16-wrap)

BF16 = mybir.dt.bfloat16
F32 = mybir.dt.float32
F8 = mybir.dt.float8e4
I16 = mybir.dt.int16

NP_F8 = ml_dtypes.float8_e4m3

_NC_CACHE = {}


def _build():
    nc = bacc.Bacc(
        "TRN2", target_bir_lowering=False, debug=False, num_swdge_queues=4
    )

    tbl = [
        [
            nc.dram_tensor(f"t{p}{h}", [HALF + 1, HIDDEN], F8, kind="ExternalInput")
            for h in range(2)
        ]
        for p in range(2)
    ]
    idxd = nc.dram_tensor(
        "idx", [128, N_ITER * IDX_COLS_ITER], I16, kind="ExternalInput"
    )
    cntd = nc.dram_tensor("cnt", [1, 16], mybir.dt.uint32, kind="ExternalInput")
    sovfd = nc.dram_tensor(
        "sovf", [N_ITER, 128, 2 * OVFN], F8, kind="ExternalInput"
    )
    i2d = nc.dram_tensor("i2", [128, 2 * 128], F8, kind="ExternalInput")
    onesd = nc.dram_tensor("ones", [1, 128], F8, kind="ExternalInput")
    b1sd = nc.dram_tensor("b1s", [1, 2 * HIDDEN], F8, kind="ExternalInput")
    vd = nc.dram_tensor("v", [128, 2, HIDDEN], F32, kind="ExternalInput")
    b2d = nc.dram_tensor("b2", [128, 1], F32, kind="ExternalInput")
    outd = nc.dram_tensor("out", [128, N_TILES], F32, kind="ExternalOutput")

    with TileContext(nc) as tc:
        with (
            tc.tile_pool(name="consts", bufs=1) as consts,
            tc.tile_pool(name="gp", bufs=BUFS) as gp,
            tc.tile_pool(name="psum", bufs=6, space="PSUM") as pp,
            tc.tile_pool(name="ep", bufs=2) as ep,
        ):
            # idx + counts go FIRST on the sync HWDGE queue so the first
            # gather isn't gated on the remaining const uploads (which go
            # out on the scalar/vector HWDGE queues in parallel).
            idxt = consts.tile([128, N_ITER, IDX_COLS_ITER], I16, tag="idx")
            nc.sync.dma_start(out=idxt[:, :, :], in_=idxd[:, :])
            # Exact overflow counts per (iter, half); trailing -1 idxs are
            # skipped by the ucode (no descriptors, no DMA); the count
            # register must match exactly.
            cnt_t = consts.tile([1, 16], mybir.dt.uint32, tag="cnt")
            nc.sync.dma_start(out=cnt_t, in_=cntd[:, :])
            cregs = []
            for g in range(16):
                r = nc.gpsimd.alloc_register(f"cnt{g}")
                nc.gpsimd.reg_load(r, cnt_t[0:1, g : g + 1])
                cregs.append(r)
            i2_t = consts.tile([128, 2, 128], F8, tag="i2")
            nc.sync.dma_start(out=i2_t, in_=i2d[:, :])
            ones_t = consts.tile([1, 128], F8, tag="ones")
            nc.sync.dma_start(out=ones_t, in_=onesd[:, :])
            b1s_t = consts.tile([1, 2, HIDDEN], F8, tag="b1s")
            nc.sync.dma_start(out=b1s_t, in_=b1sd[:, :])
            v_t = consts.tile([128, 2, HIDDEN], F32, tag="v")
            nc.scalar.dma_start(out=v_t, in_=vd[:, :, :])
            b2_t = consts.tile([128, 1], F32, tag="b2")
            nc.scalar.dma_start(out=b2_t, in_=b2d[:, :])
            outst = consts.tile([128, N_TILES], F32, tag="outst")
            sovf_t = consts.tile([128, N_ITER, 2, 2, 128], F8, tag="sovf")
            for i in range(N_ITER):
                nc.scalar.dma_start(out=sovf_t[:, i, :, :, :], in_=sovfd[i, :, :])

            acc0 = None
            for i in range(N_ITER):
                t, p = i // 2, i % 2
                # Gathers: per half 2 chunks of 1024 (8 blocks each, always
                # full: identity lanes pad with the zero row), then per half
                # one 256-idx overflow chunk with exact count. Queue map is
                # iteration-stationary: q0: h0c0+ovf0, q1: h0c1, q2:
                # h1c0+ovf1, q3: h1c1.
                gts = []
                for h in range(2):
                    for cch in range(2):
                        gt = gp.tile([128, 8, HIDDEN], F8, tag=f"g{h}{cch}")
                        c0 = (2 * h + cch) * 64
                        nc.gpsimd.dma_gather(
                            gt,
                            tbl[p][h][:, :],
                            idxt[:, i, c0 : c0 + 64],
                            1024,
                            1024,
                            HIDDEN,
                            single_packet=SINGLE_PACKET,
                            queue_num=2 * h + cch,
                        )
                        gts.append(gt)
                ovf = []
                for h in range(2):
                    ot = gp.tile([128, OVFN // 128, HIDDEN], F8, tag=f"o{h}")
                    c0 = 4 * 64 + h * (OVFN // 16)
                    nreg = OVFN if i < BUFS else cregs[2 * i + h]
                    nc.gpsimd.dma_gather(
                        ot,
                        tbl[p][h][:, :],
                        idxt[:, i, c0 : c0 + OVFN // 16],
                        OVFN,
                        nreg,
                        HIDDEN,
                        single_packet=SINGLE_PACKET,
                        queue_num=2 * h,
                    )
                    ovf.append(ot)

                psum = pp.tile([128, HIDDEN], F32, tag="psum")
                # l1 bias as a rank-1 matmul: ones[1,128].T @ b1s[1,256].
                nc.tensor.matmul(
                    psum,
                    lhsT=ones_t[0:1, :],
                    rhs=b1s_t[0:1, p, :],
                    start=True,
                    stop=False,
                )
                # Identity DoubleRow matmuls: psum[r,:] += G[r,2c,:]+G[r,2c+1,:]
                for gt in gts:
                    for c2 in range(4):
                        nc.tensor.matmul(
                            psum,
                            lhsT=i2_t[:, :, :],
                            rhs=gt[:, 2 * c2 : 2 * c2 + 2, :],
                            perf_mode=mybir.MatmulPerfMode.DoubleRow,
                            start=False,
                            stop=False,
                        )
                # Overflow: small one-hot S per (iter, half).
                for h in range(2):
                    nc.tensor.matmul(
                        psum,
                        lhsT=sovf_t[:, i, h, :, :],
                        rhs=ovf[h][:, :, :],
                        perf_mode=mybir.MatmulPerfMode.DoubleRow,
                        start=False,
                        stop=(h == 1),
                    )

                # Epilogue: relu off PSUM, fused (min SCALE, * v/SCALE),
                # reduce; combine perspectives + b2.
                clr = ep.tile([128, HIDDEN], F32, tag="clr")
                nc.vector.tensor_scalar_max(clr, psum, 0.0)
                prod = ep.tile([128, HIDDEN], F32, tag="prod")
                nc.vector.scalar_tensor_tensor(
                    prod,
                    clr,
                    SCALE,
                    v_t[:, p, :],
                    op0=mybir.AluOpType.min,
                    op1=mybir.AluOpType.mult,
                )
                if p == 0:
                    acc0 = ep.tile([128, 1], F32, tag="acc0")
                    nc.vector.tensor_reduce(
                        acc0, prod, axis=mybir.AxisListType.X, op=mybir.AluOpType.add
                    )
                else:
                    acc1 = ep.tile([128, 1], F32, tag="acc1")
                    nc.vector.tensor_reduce(
                        acc1, prod, axis=mybir.AxisListType.X, op=mybir.AluOpType.add
                    )
                    # out[:, t] = (acc0 + b2) + acc1
                    nc.vector.scalar_tensor_tensor(
                        outst[:, t : t + 1],
                        acc0,
                        b2_t,
                        acc1,
                        op0=mybir.AluOpType.add,
                        op1=mybir.AluOpType.add,
                    )
            nc.sync.dma_start(out=outd[:, :], in_=outst)

    nc.compile()
    return nc


def _wrap16(v):
    """Linear idx vector -> [16, n/16] SWDGE wrap, tiled to 128 partitions."""
    n = len(v)
    return np.tile(v.reshape(n // 16, 16).T, (8, 1))  # [128, n//16]


def _prep(x1, x2, l1_weights, l1_biases, l2_weight, l2_bias):
    """Host-side: fp8 tables, per-core identity-lane index lists, overflow
    S matrices, epilogue constants."""
    wt = l1_weights.astype(np.float32).transpose(0, 2, 1)  # [2, I, H]
    tabs = {}
    for p in range(2):
        for h in range(2):
            tt = np.zeros((HALF + 1, HIDDEN), dtype=NP_F8)
            tt[:HALF] = (wt[p, h * HALF : (h + 1) * HALF] * SCALE).astype(NP_F8)
            tabs[f"t{p}{h}"] = tt

    i2 = np.zeros((128, 2, 128), NP_F8)
    for tcol in range(2):
        i2[np.arange(128), tcol, np.arange(128)] = 1.0
    ones = np.ones((1, 128), NP_F8)
    b1s = (l1_biases.astype(np.float32).reshape(1, 2 * HIDDEN) * SCALE).astype(
        NP_F8
    )
    v_full = np.ascontiguousarray(
        np.broadcast_to(
            (l2_weight.astype(np.float32) / SCALE).reshape(1, 2, HIDDEN),
            (128, 2, HIDDEN),
        )
    )
    b2_full = np.full((128, 1), float(np.asarray(l2_bias).reshape(-1)[0]), np.float32)

    xs = [np.asarray(x1), np.asarray(x2)]
    in_maps = []
    for c in range(N_CORES):
        idx_arr = np.empty((N_ITER, 128, IDX_COLS_ITER), np.int16)
        sovf = np.zeros((N_ITER, 128, 2, 2, 128), NP_F8)
        cnt = np.zeros((1, 16), np.uint32)
        for i in range(N_ITER):
            t, p = i // 2, i % 2
            blk = xs[p][c * B_CORE + t * 128 : c * B_CORE + (t + 1) * 128]
            r_all, f_all = np.nonzero(blk)
            cols = []
            for h in range(2):
                sel = (f_all >= h * HALF) & (f_all < (h + 1) * HALF)
                r, f = r_all[sel], f_all[sel] - h * HALF
                # identity lanes: row r's first B_ID features at blocks 0..,
                # lane r; rest to overflow
                blk_idx = np.full((2 * 8, 128), ZR, np.int16)
                pos = np.zeros(128, np.int64)
                ovf_f, ovf_r = [], []
                for rr, ff in zip(r, f):
                    if pos[rr] < B_ID:
                        blk_idx[pos[rr], rr] = ff
                        pos[rr] += 1
                    else:
                        ovf_f.append(ff)
                        ovf_r.append(rr)
                cols.append(_wrap16(blk_idx[:8].reshape(1024)))
                cols.append(_wrap16(blk_idx[8:].reshape(1024)))
                m = len(ovf_f)
                assert m <= OVFN, (c, i, h, m)
                if i < BUFS:
                    ov = np.full(OVFN, ZR, np.int16)
                    cnt[0, 2 * i + h] = OVFN
                else:
                    ov = np.full(OVFN, -1, np.int16)
                    cnt[0, 2 * i + h] = max(m, 1)
                    if m == 0:
                        ov[0] = ZR
                ov[:m] = ovf_f
                j = np.arange(m)
                sovf[i, j % 128, h, j // 128, ovf_r] = 1.0
                cols.append(_wrap16(ov))
            # order: h0c0, h0c1, h1c0, h1c1, ovf0, ovf1
            idx_arr[i] = np.concatenate(
                [cols[0], cols[1], cols[3], cols[4], cols[2], cols[5]], axis=1
            )
        in_map = dict(tabs)
        in_map.update(
            idx=np.ascontiguousarray(
                idx_arr.transpose(1, 0, 2).reshape(128, N_ITER * IDX_COLS_ITER)
            ),
            cnt=cnt,
            sovf=np.ascontiguousarray(sovf.reshape(N_ITER, 128, 2 * OVFN)),
            i2=np.ascontiguousarray(i2.reshape(128, 2 * 128)),
            ones=ones,
            b1s=b1s,
            v=v_full,
            b2=b2_full,
        )
        in_maps.append(in_map)
    return in_maps


def _run(x1, x2, l1_weights, l1_biases, l2_weight, l2_bias, trace=False):
    in_maps = _prep(x1, x2, l1_weights, l1_biases, l2_weight, l2_bias)
    if "nc" not in _NC_CACHE:
        _NC_CACHE["nc"] = _build()
    nc = _NC_CACHE["nc"]

    res = run_bass_kernel_spmd(
        nc, in_maps, core_ids=list(range(N_CORES)), trace=trace
    )
    out = np.concatenate(
        [
            np.ascontiguousarray(res.results[c]["out"].T).reshape(B_CORE)
            for c in range(N_CORES)
        ]
    )
    return out.astype(np.float32), res


def kernel(**inputs):
    out, _ = _run(**inputs)
    return out


def kernel_profiled(**inputs):
    _, res = _run(**inputs, trace=True)
    return res


# revision 6
# speedup vs baseline: 1.0599x; 1.0217x over previous
"""NNUE forward kernel for Trainium2, 8-core SPMD, batch-sharded,
sparsity-exploiting (embedding-gather formulation), fp8 tables +
identity-matrix reduction.

Reference computation (B=4096, I=40960, H=256):
    h_p = clip(x_p @ W_p.T + b_p, 0, 1)   for p in {1,2}
    out = concat(h1, h2) @ v + b2         -> (B,)

x_p rows are sparse binary (~30 active features of 40960), so
x_p @ W_p.T is an embedding-sum: h[b] = sum_{active f} W_p.T[f, :].

Tables are fp8 e4m3, pre-scaled by 2^15 so values sit in e4m3's normal
range (weights are ~U(-1/202, 1/202)); the scale folds into the
epilogue: h = clip(psum, 0, SCALE) * (v / SCALE). Measured end-to-end
norm-rel error of e4m3 quantization on this data: 5.8e-3 (tolerance
2e-2).

Identity-matrix reduction: the host assigns each batch row a fixed
budget of B_ID=16 gather lanes per table half: the j-th gathered slot
of a block lands on SBUF partition j%128, and we place row r's
features on lane r. The PE reduction over each pair of 128-slot blocks
is then lhsT = a constant [128,2,128] identity (DoubleRow fp8 matmul,
2x rate), so no per-iteration selector-matrix upload is needed. Rows
with more than 16 features per half spill to a small overflow gather
(<=2 blocks of 128) reduced with a tiny uploaded one-hot S. The l1
bias enters the PSUM group as a rank-1 matmul (ones[1,128] x
b1s[1,256]). Unused identity lanes gather a zero table row; overflow
padding uses trailing -1 indices (skipped by the SWDGE ucode) with an
exact per-(iter,half) count register, except the first BUFS iterations
whose fresh SBUF buffers must be fully overwritten to avoid NaN
garbage under the S=0 columns.

Per core (512 batch rows): 8 iterations over (128-row tile,
perspective); per iteration 6 dma_gathers (2x 1024-idx per half +
2x 256-idx overflow; <=1024 idxs per call -- the SWDGE ucode wedges
beyond that; 4 SWDGE queues round-robin), 8 identity DoubleRow
matmuls + 2 overflow DoubleRow + 1 bias matmul into PSUM, then a
Vector epilogue (relu off PSUM, fused min/dot with v/SCALE, + b2).
No collectives (pure data parallel; batch-sharded).
"""

import numpy as np
import ml_dtypes

import concourse.bass as bass
import concourse.mybir as mybir
from concourse import bacc
from concourse.tile import TileContext
from concourse.bass_utils import run_bass_kernel_spmd

BATCH = 4096
INPUT_SIZE = 40960
HIDDEN = 256
N_CORES = 8
B_CORE = BATCH // N_CORES  # 512
N_TILES = B_CORE // 128  # 4
HALF = INPUT_SIZE // 2  # 20480 rows per table half (int16 index range)
ZR = HALF  # zero row index (padding target)

SCALE = 2.0**15  # fp8 pre-scale; folded into epilogue clip + v
B_ID = 16  # identity-lane budget per row per half
OVFN = 256  # overflow slots per (tile, persp, half): 2 blocks of 128
BUFS = 4  # gather pool depth; first BUFS iterations fully pad
SINGLE_PACKET = False  # let the SWDGE ucode packetize 256B descriptors
N_ITER = 2 * N_TILES  # 8 (tile, persp) iterations
IDX_COLS_ITER = 4 * 64 + 2 * (OVFN // 16)  # idx cols per iter (int16 16-wrap)

BF16 = mybir.dt.bfloat16
F32 = mybir.dt.float32
F8 = mybir.dt.float8e4
I16 = mybir.dt.int16

NP_F8 = ml_dtypes.float8_e4m3

_NC_CACHE = {}


def _build():
    nc = bacc.Bacc(
        "TRN2",
        target_bir_lowering=False,
        debug=False,
        num_swdge_queues=4,
        # 4096-descriptor SWDGE carveout (default 16384B = 1024 descs = ONE
        # 1024-idx gather): lets descriptor generation run ahead of SDMA
        # drain instead of lockstepping gen->drain per gather.
        dynamic_dma_scratch_size=65536,
    )

    tbl = [
        [
            nc.dram_tensor(f"t{p}{h}", [HALF + 1, HIDDEN], F8, kind="ExternalInput")
            for h in range(2)
        ]
        for p in range(2)
    ]
    idxd = nc.dram_tensor(
        "idx", [128, N_ITER * IDX_COLS_ITER], I16, kind="ExternalInput"
    )
    cntd = nc.dram_tensor("cnt", [1, 16], mybir.dt.uint32, kind="ExternalInput")
    sovfd = nc.dram_tensor(
        "sovf", [N_ITER, 128, 2 * OVFN], F8, kind="ExternalInput"
    )
    i2d = nc.dram_tensor("i2", [128, 2 * 128], F8, kind="ExternalInput")
    onesd = nc.dram_tensor("ones", [1, 128], F8, kind="ExternalInput")
    b1sd = nc.dram_tensor("b1s", [1, 2 * HIDDEN], F8, kind="ExternalInput")
    vd = nc.dram_tensor("v", [128, 2, HIDDEN], F32, kind="ExternalInput")
    b2d = nc.dram_tensor("b2", [128, 1], F32, kind="ExternalInput")
    outd = nc.dram_tensor("out", [128, N_TILES], F32, kind="ExternalOutput")

    with TileContext(nc) as tc:
        with (
            tc.tile_pool(name="consts", bufs=1) as consts,
            tc.tile_pool(name="gp", bufs=BUFS) as gp,
            tc.tile_pool(name="psum", bufs=6, space="PSUM") as pp,
            tc.tile_pool(name="ep", bufs=2) as ep,
        ):
            # idx + counts go FIRST on the sync HWDGE queue so the first
            # gather isn't gated on the remaining const uploads (which go
            # out on the scalar/vector HWDGE queues in parallel).
            idxt = consts.tile([128, N_ITER, IDX_COLS_ITER], I16, tag="idx")
            nc.sync.dma_start(out=idxt[:, :, :], in_=idxd[:, :])
            # Exact overflow counts per (iter, half); trailing -1 idxs are
            # skipped by the ucode (no descriptors, no DMA); the count
            # register must match exactly.
            cnt_t = consts.tile([1, 16], mybir.dt.uint32, tag="cnt")
            nc.sync.dma_start(out=cnt_t, in_=cntd[:, :])
            cregs = []
            for g in range(16):
                r = nc.gpsimd.alloc_register(f"cnt{g}")
                nc.gpsimd.reg_load(r, cnt_t[0:1, g : g + 1])
                cregs.append(r)
            i2_t = consts.tile([128, 2, 128], F8, tag="i2")
            nc.sync.dma_start(out=i2_t, in_=i2d[:, :])
            ones_t = consts.tile([1, 128], F8, tag="ones")
            nc.sync.dma_start(out=ones_t, in_=onesd[:, :])
            b1s_t = consts.tile([1, 2, HIDDEN], F8, tag="b1s")
            nc.sync.dma_start(out=b1s_t, in_=b1sd[:, :])
            v_t = consts.tile([128, 2, HIDDEN], F32, tag="v")
            nc.scalar.dma_start(out=v_t, in_=vd[:, :, :])
            b2_t = consts.tile([128, 1], F32, tag="b2")
            nc.scalar.dma_start(out=b2_t, in_=b2d[:, :])
            outst = consts.tile([128, N_TILES], F32, tag="outst")
            sovf_t = consts.tile([128, N_ITER, 2, 2, 128], F8, tag="sovf")
            for i in range(N_ITER):
                nc.scalar.dma_start(out=sovf_t[:, i, :, :, :], in_=sovfd[i, :, :])

            acc0 = None
            for i in range(N_ITER):
                t, p = i // 2, i % 2
                # Gathers: per half 2 chunks of 1024 (8 blocks each, always
                # full: identity lanes pad with the zero row), then per half
                # one 256-idx overflow chunk with exact count. Queue map is
                # iteration-stationary: q0: h0c0+ovf0, q1: h0c1, q2:
                # h1c0+ovf1, q3: h1c1.
                gts = []
                for h in range(2):
                    for cch in range(2):
                        gt = gp.tile([128, 8, HIDDEN], F8, tag=f"g{h}{cch}")
                        c0 = (2 * h + cch) * 64
                        nc.gpsimd.dma_gather(
                            gt,
                            tbl[p][h][:, :],
                            idxt[:, i, c0 : c0 + 64],
                            1024,
                            1024,
                            HIDDEN,
                            single_packet=SINGLE_PACKET,
                            queue_num=2 * h + cch,
                        )
                        gts.append(gt)
                ovf = []
                for h in range(2):
                    ot = gp.tile([128, OVFN // 128, HIDDEN], F8, tag=f"o{h}")
                    c0 = 4 * 64 + h * (OVFN // 16)
                    nreg = OVFN if i < BUFS else cregs[2 * i + h]
                    nc.gpsimd.dma_gather(
                        ot,
                        tbl[p][h][:, :],
                        idxt[:, i, c0 : c0 + OVFN // 16],
                        OVFN,
                        nreg,
                        HIDDEN,
                        single_packet=SINGLE_PACKET,
                        queue_num=2 * h,
                    )
                    ovf.append(ot)

                psum = pp.tile([128, HIDDEN], F32, tag="psum")
                # l1 bias as a rank-1 matmul: ones[1,128].T @ b1s[1,256].
                nc.tensor.matmul(
                    psum,
                    lhsT=ones_t[0:1, :],
                    rhs=b1s_t[0:1, p, :],
                    start=True,
                    stop=False,
                )
                # Identity DoubleRow matmuls: psum[r,:] += G[r,2c,:]+G[r,2c+1,:]
                for gt in gts:
                    for c2 in range(4):
                        nc.tensor.matmul(
                            psum,
                            lhsT=i2_t[:, :, :],
                            rhs=gt[:, 2 * c2 : 2 * c2 + 2, :],
                            perf_mode=mybir.MatmulPerfMode.DoubleRow,
                            start=False,
                            stop=False,
                        )
                # Overflow: small one-hot S per (iter, half).
                for h in range(2):
                    nc.tensor.matmul(
                        psum,
                        lhsT=sovf_t[:, i, h, :, :],
                        rhs=ovf[h][:, :, :],
                        perf_mode=mybir.MatmulPerfMode.DoubleRow,
                        start=False,
                        stop=(h == 1),
                    )

                # Epilogue: relu off PSUM, fused (min SCALE, * v/SCALE),
                # reduce; combine perspectives + b2.
                clr = ep.tile([128, HIDDEN], F32, tag="clr")
                nc.vector.tensor_scalar_max(clr, psum, 0.0)
                prod = ep.tile([128, HIDDEN], F32, tag="prod")
                nc.vector.scalar_tensor_tensor(
                    prod,
                    clr,
                    SCALE,
                    v_t[:, p, :],
                    op0=mybir.AluOpType.min,
                    op1=mybir.AluOpType.mult,
                )
                if p == 0:
                    acc0 = ep.tile([128, 1], F32, tag="acc0")
                    nc.vector.tensor_reduce(
                        acc0, prod, axis=mybir.AxisListType.X, op=mybir.AluOpType.add
                    )
                else:
                    acc1 = ep.tile([128, 1], F32, tag="acc1")
                    nc.vector.tensor_reduce(
                        acc1, prod, axis=mybir.AxisListType.X, op=mybir.AluOpType.add
                    )
                    # out[:, t] = (acc0 + b2) + acc1
                    nc.vector.scalar_tensor_tensor(
                        outst[:, t : t + 1],
                        acc0,
                        b2_t,
                        acc1,
                        op0=mybir.AluOpType.add,
                        op1=mybir.AluOpType.add,
                    )
            nc.sync.dma_start(out=outd[:, :], in_=outst)

    nc.compile()
    return nc


def _wrap16(v):
    """Linear idx vector -> [16, n/16] SWDGE wrap, tiled to 128 partitions."""
    n = len(v)
    return np.tile(v.reshape(n // 16, 16).T, (8, 1))  # [128, n//16]


def _prep(x1, x2, l1_weights, l1_biases, l2_weight, l2_bias):
    """Host-side: fp8 tables, per-core identity-lane index lists, overflow
    S matrices, epilogue constants."""
    wt = l1_weights.astype(np.float32).transpose(0, 2, 1)  # [2, I, H]
    tabs = {}
    for p in range(2):
        for h in range(2):
            tt = np.zeros((HALF + 1, HIDDEN), dtype=NP_F8)
            tt[:HALF] = (wt[p, h * HALF : (h + 1) * HALF] * SCALE).astype(NP_F8)
            tabs[f"t{p}{h}"] = tt

    i2 = np.zeros((128, 2, 128), NP_F8)
    for tcol in range(2):
        i2[np.arange(128), tcol, np.arange(128)] = 1.0
    ones = np.ones((1, 128), NP_F8)
    b1s = (l1_biases.astype(np.float32).reshape(1, 2 * HIDDEN) * SCALE).astype(
        NP_F8
    )
    v_full = np.ascontiguousarray(
        np.broadcast_to(
            (l2_weight.astype(np.float32) / SCALE).reshape(1, 2, HIDDEN),
            (128, 2, HIDDEN),
        )
    )
    b2_full = np.full((128, 1), float(np.asarray(l2_bias).reshape(-1)[0]), np.float32)

    xs = [np.asarray(x1), np.asarray(x2)]
    in_maps = []
    for c in range(N_CORES):
        idx_arr = np.empty((N_ITER, 128, IDX_COLS_ITER), np.int16)
        sovf = np.zeros((N_ITER, 128, 2, 2, 128), NP_F8)
        cnt = np.zeros((1, 16), np.uint32)
        for i in range(N_ITER):
            t, p = i // 2, i % 2
            blk = xs[p][c * B_CORE + t * 128 : c * B_CORE + (t + 1) * 128]
            r_all, f_all = np.nonzero(blk)
            cols = []
            for h in range(2):
                sel = (f_all >= h * HALF) & (f_all < (h + 1) * HALF)
                r, f = r_all[sel], f_all[sel] - h * HALF
                # identity lanes: row r's first B_ID features at blocks 0..,
                # lane r; rest to overflow
                blk_idx = np.full((2 * 8, 128), ZR, np.int16)
                pos = np.zeros(128, np.int64)
                ovf_f, ovf_r = [], []
                for rr, ff in zip(r, f):
                    if pos[rr] < B_ID:
                        blk_idx[pos[rr], rr] = ff
                        pos[rr] += 1
                    else:
                        ovf_f.append(ff)
                        ovf_r.append(rr)
                cols.append(_wrap16(blk_idx[:8].reshape(1024)))
                cols.append(_wrap16(blk_idx[8:].reshape(1024)))
                m = len(ovf_f)
                assert m <= OVFN, (c, i, h, m)
                if i < BUFS:
                    ov = np.full(OVFN, ZR, np.int16)
                    cnt[0, 2 * i + h] = OVFN
                else:
                    ov = np.full(OVFN, -1, np.int16)
                    cnt[0, 2 * i + h] = max(m, 1)
                    if m == 0:
                        ov[0] = ZR
                ov[:m] = ovf_f
                j = np.arange(m)
                sovf[i, j % 128, h, j // 128, ovf_r] = 1.0
                cols.append(_wrap16(ov))
            # order: h0c0, h0c1, h1c0, h1c1, ovf0, ovf1
            idx_arr[i] = np.concatenate(
                [cols[0], cols[1], cols[3], cols[4], cols[2], cols[5]], axis=1
            )
        in_map = dict(tabs)
        in_map.update(
            idx=np.ascontiguousarray(
                idx_arr.transpose(1, 0, 2).reshape(128, N_ITER * IDX_COLS_ITER)
            ),
            cnt=cnt,
            sovf=np.ascontiguousarray(sovf.reshape(N_ITER, 128, 2 * OVFN)),
            i2=np.ascontiguousarray(i2.reshape(128, 2 * 128)),
            ones=ones,
            b1s=b1s,
            v=v_full,
            b2=b2_full,
        )
        in_maps.append(in_map)
    return in_maps


def _run(x1, x2, l1_weights, l1_biases, l2_weight, l2_bias, trace=False):
    in_maps = _prep(x1, x2, l1_weights, l1_biases, l2_weight, l2_bias)
    if "nc" not in _NC_CACHE:
        _NC_CACHE["nc"] = _build()
    nc = _NC_CACHE["nc"]

    res = run_bass_kernel_spmd(
        nc, in_maps, core_ids=list(range(N_CORES)), trace=trace
    )
    out = np.concatenate(
        [
            np.ascontiguousarray(res.results[c]["out"].T).reshape(B_CORE)
            for c in range(N_CORES)
        ]
    )
    return out.astype(np.float32), res


def kernel(**inputs):
    out, _ = _run(**inputs)
    return out


def kernel_profiled(**inputs):
    _, res = _run(**inputs, trace=True)
    return res


# revision 7
# speedup vs baseline: 1.0891x; 1.0275x over previous
"""NNUE forward kernel for Trainium2, 8-core SPMD, batch-sharded,
sparsity-exploiting (embedding-gather formulation), fp8 tables +
identity-matrix reduction.

Reference computation (B=4096, I=40960, H=256):
    h_p = clip(x_p @ W_p.T + b_p, 0, 1)   for p in {1,2}
    out = concat(h1, h2) @ v + b2         -> (B,)

x_p rows are sparse binary (~30 active features of 40960), so
x_p @ W_p.T is an embedding-sum: h[b] = sum_{active f} W_p.T[f, :].

Tables are fp8 e4m3, pre-scaled by 2^15 so values sit in e4m3's normal
range (weights are ~U(-1/202, 1/202)); the scale folds into the
epilogue: h = clip(psum, 0, SCALE) * (v / SCALE). Measured end-to-end
norm-rel error of e4m3 quantization on this data: 5.9e-3 (tolerance
2e-2).

Identity-matrix reduction: the host assigns each batch row a fixed
budget of B_ID=16 gather lanes per table half: the j-th gathered slot
of a block lands on SBUF partition j%128, and we place row r's
features on lane r. The PE reduction over each pair of 128-slot blocks
is then lhsT = a constant [128,2,128] identity (DoubleRow fp8 matmul,
2x rate), so no per-iteration selector-matrix upload is needed. Rows
with more than 16 features per half spill to a small overflow space
reduced with a tiny uploaded one-hot S. The l1 bias enters the PSUM
group as a rank-1 matmul (ones[1,128] x b1s[1,256]). Unused lanes
gather a zero table row, so no SBUF slot ever holds garbage.

SWDGE gather instructions serialize on the gpsimd engine (~1.3us+
each: 994ns fixed + ring-drain lockstep), so the kernel minimizes
gather COUNT: all overflow slots for all 8 iterations are fetched by 4
upfront 1024-idx gathers (one per (persp, half) table, 2 blocks per
128-row tile), leaving the steady-state loop at exactly 4 1024-idx
gathers per (tile, persp) iteration on the 4 SWDGE queues
(single_packet=False halves SDMA per-descriptor drain cost vs
single-packet mode). Per iteration the PE does 16 identity DoubleRow
matmuls + 2 overflow DoubleRow + 1 bias matmul into PSUM; the Vector
epilogue (relu off PSUM, fused min/dot with v/SCALE, + b2) follows.
No collectives (pure data parallel; batch-sharded).
"""

import numpy as np
import ml_dtypes

import concourse.bass as bass
import concourse.mybir as mybir
from concourse import bacc
from concourse.tile import TileContext
from concourse.bass_utils import run_bass_kernel_spmd

BATCH = 4096
INPUT_SIZE = 40960
HIDDEN = 256
N_CORES = 8
B_CORE = BATCH // N_CORES  # 512
N_TILES = B_CORE // 128  # 4
HALF = INPUT_SIZE // 2  # 20480 rows per table half (int16 index range)
ZR = HALF  # zero row index (padding target)

SCALE = 2.0**15  # fp8 pre-scale; folded into epilogue clip + v
B_ID = 16  # identity-lane budget per row per half
OVFB = 2  # overflow blocks of 128 per (tile, persp, half)
BUFS = 4  # gather pool depth
N_ITER = 2 * N_TILES  # 8 (tile, persp) iterations
IDX_COLS_ITER = 4 * 64  # idx cols per iter (int16 16-wrap)
IDX_COLS_OVF = 4 * 64  # upfront overflow idx cols (4 tables x 1024 idx)

BF16 = mybir.dt.bfloat16
F32 = mybir.dt.float32
F8 = mybir.dt.float8e4
I16 = mybir.dt.int16

NP_F8 = ml_dtypes.float8_e4m3

_NC_CACHE = {}


def _build():
    nc = bacc.Bacc(
        "TRN2",
        target_bir_lowering=False,
        debug=False,
        num_swdge_queues=4,
        dynamic_dma_scratch_size=65536,
    )

    tbl = [
        [
            nc.dram_tensor(f"t{p}{h}", [HALF + 1, HIDDEN], F8, kind="ExternalInput")
            for h in range(2)
        ]
        for p in range(2)
    ]
    idxd = nc.dram_tensor(
        "idx",
        [128, IDX_COLS_OVF + N_ITER * IDX_COLS_ITER],
        I16,
        kind="ExternalInput",
    )
    sovfd = nc.dram_tensor(
        "sovf", [N_ITER, 128, 2 * OVFB * 128], F8, kind="ExternalInput"
    )
    i2d = nc.dram_tensor("i2", [128, 2 * 128], F8, kind="ExternalInput")
    onesd = nc.dram_tensor("ones", [1, 128], F8, kind="ExternalInput")
    b1sd = nc.dram_tensor("b1s", [1, 2 * HIDDEN], F8, kind="ExternalInput")
    vd = nc.dram_tensor("v", [128, 2, HIDDEN], F32, kind="ExternalInput")
    b2d = nc.dram_tensor("b2", [128, 1], F32, kind="ExternalInput")
    outd = nc.dram_tensor("out", [128, N_TILES], F32, kind="ExternalOutput")

    with TileContext(nc) as tc:
        with (
            tc.tile_pool(name="consts", bufs=1) as consts,
            tc.tile_pool(name="gp", bufs=BUFS) as gp,
            tc.tile_pool(name="psum", bufs=6, space="PSUM") as pp,
            tc.tile_pool(name="ep", bufs=2) as ep,
        ):
            # idx goes FIRST on the sync HWDGE queue so the upfront gathers
            # aren't gated on the remaining const uploads (which go out on
            # the scalar HWDGE queue in parallel).
            idxt = consts.tile(
                [128, IDX_COLS_OVF + N_ITER * IDX_COLS_ITER], I16, tag="idx"
            )
            nc.sync.dma_start(out=idxt[:, :], in_=idxd[:, :])
            i2_t = consts.tile([128, 2, 128], F8, tag="i2")
            nc.sync.dma_start(out=i2_t, in_=i2d[:, :])
            ones_t = consts.tile([1, 128], F8, tag="ones")
            nc.sync.dma_start(out=ones_t, in_=onesd[:, :])
            b1s_t = consts.tile([1, 2, HIDDEN], F8, tag="b1s")
            nc.sync.dma_start(out=b1s_t, in_=b1sd[:, :])
            v_t = consts.tile([128, 2, HIDDEN], F32, tag="v")
            nc.scalar.dma_start(out=v_t, in_=vd[:, :, :])
            b2_t = consts.tile([128, 1], F32, tag="b2")
            nc.scalar.dma_start(out=b2_t, in_=b2d[:, :])
            outst = consts.tile([128, N_TILES], F32, tag="outst")
            sovf_t = consts.tile([128, N_ITER, 2, OVFB, 128], F8, tag="sovf")
            for i in range(N_ITER):
                nc.scalar.dma_start(out=sovf_t[:, i, :, :, :], in_=sovfd[i, :, :])

            # Upfront overflow gathers: one 1024-idx gather per (persp,
            # half) table, 2 blocks per tile, fully ZR-padded (no count
            # registers, no garbage).
            ovft = []
            for p in range(2):
                row = []
                for h in range(2):
                    ot = consts.tile(
                        [128, N_TILES * OVFB, HIDDEN], F8, tag=f"ovf{p}{h}"
                    )
                    c0 = (2 * p + h) * 64
                    nc.gpsimd.dma_gather(
                        ot,
                        tbl[p][h][:, :],
                        idxt[:, c0 : c0 + 64],
                        N_TILES * OVFB * 128,
                        N_TILES * OVFB * 128,
                        HIDDEN,
                        single_packet=False,
                        queue_num=2 * p + h,
                    )
                    row.append(ot)
                ovft.append(row)

            acc0 = None
            for i in range(N_ITER):
                t, p = i // 2, i % 2
                ib = IDX_COLS_OVF + i * IDX_COLS_ITER
                # Per half 2 chunks of 1024 (8 blocks each, always full:
                # identity lanes pad with the zero row). Queue map is
                # iteration-stationary.
                gts = []
                for h in range(2):
                    for cch in range(2):
                        gt = gp.tile([128, 8, HIDDEN], F8, tag=f"g{h}{cch}")
                        c0 = ib + (2 * h + cch) * 64
                        nc.gpsimd.dma_gather(
                            gt,
                            tbl[p][h][:, :],
                            idxt[:, c0 : c0 + 64],
                            1024,
                            1024,
                            HIDDEN,
                            single_packet=False,
                            queue_num=2 * h + cch,
                        )
                        gts.append(gt)

                psum = pp.tile([128, HIDDEN], F32, tag="psum")
                # l1 bias as a rank-1 matmul: ones[1,128].T @ b1s[1,256].
                nc.tensor.matmul(
                    psum,
                    lhsT=ones_t[0:1, :],
                    rhs=b1s_t[0:1, p, :],
                    start=True,
                    stop=False,
                )
                # Identity DoubleRow matmuls: psum[r,:] += G[r,2c,:]+G[r,2c+1,:]
                for gt in gts:
                    for c2 in range(4):
                        nc.tensor.matmul(
                            psum,
                            lhsT=i2_t[:, :, :],
                            rhs=gt[:, 2 * c2 : 2 * c2 + 2, :],
                            perf_mode=mybir.MatmulPerfMode.DoubleRow,
                            start=False,
                            stop=False,
                        )
                # Overflow: small one-hot S per (iter, half) against the
                # upfront-gathered overflow tile (this tile's 2 blocks).
                for h in range(2):
                    nc.tensor.matmul(
                        psum,
                        lhsT=sovf_t[:, i, h, :, :],
                        rhs=ovft[p][h][:, OVFB * t : OVFB * t + 2, :],
                        perf_mode=mybir.MatmulPerfMode.DoubleRow,
                        start=False,
                        stop=(h == 1),
                    )

                # Epilogue: relu off PSUM, fused (min SCALE, * v/SCALE),
                # reduce; combine perspectives + b2.
                clr = ep.tile([128, HIDDEN], F32, tag="clr")
                nc.vector.tensor_scalar_max(clr, psum, 0.0)
                prod = ep.tile([128, HIDDEN], F32, tag="prod")
                nc.vector.scalar_tensor_tensor(
                    prod,
                    clr,
                    SCALE,
                    v_t[:, p, :],
                    op0=mybir.AluOpType.min,
                    op1=mybir.AluOpType.mult,
                )
                if p == 0:
                    acc0 = ep.tile([128, 1], F32, tag="acc0")
                    nc.vector.tensor_reduce(
                        acc0, prod, axis=mybir.AxisListType.X, op=mybir.AluOpType.add
                    )
                else:
                    acc1 = ep.tile([128, 1], F32, tag="acc1")
                    nc.vector.tensor_reduce(
                        acc1, prod, axis=mybir.AxisListType.X, op=mybir.AluOpType.add
                    )
                    # out[:, t] = (acc0 + b2) + acc1
                    nc.vector.scalar_tensor_tensor(
                        outst[:, t : t + 1],
                        acc0,
                        b2_t,
                        acc1,
                        op0=mybir.AluOpType.add,
                        op1=mybir.AluOpType.add,
                    )
            nc.sync.dma_start(out=outd[:, :], in_=outst)

    nc.compile()
    return nc


def _wrap16(v):
    """Linear idx vector -> [16, n/16] SWDGE wrap, tiled to 128 partitions."""
    n = len(v)
    return np.tile(v.reshape(n // 16, 16).T, (8, 1))  # [128, n//16]


def _prep(x1, x2, l1_weights, l1_biases, l2_weight, l2_bias):
    """Host-side: fp8 tables, per-core identity-lane index lists, overflow
    S matrices, epilogue constants."""
    wt = l1_weights.astype(np.float32).transpose(0, 2, 1)  # [2, I, H]
    tabs = {}
    for p in range(2):
        for h in range(2):
            tt = np.zeros((HALF + 1, HIDDEN), dtype=NP_F8)
            tt[:HALF] = (wt[p, h * HALF : (h + 1) * HALF] * SCALE).astype(NP_F8)
            tabs[f"t{p}{h}"] = tt

    i2 = np.zeros((128, 2, 128), NP_F8)
    for tcol in range(2):
        i2[np.arange(128), tcol, np.arange(128)] = 1.0
    ones = np.ones((1, 128), NP_F8)
    b1s = (l1_biases.astype(np.float32).reshape(1, 2 * HIDDEN) * SCALE).astype(
        NP_F8
    )
    v_full = np.ascontiguousarray(
        np.broadcast_to(
            (l2_weight.astype(np.float32) / SCALE).reshape(1, 2, HIDDEN),
            (128, 2, HIDDEN),
        )
    )
    b2_full = np.full((128, 1), float(np.asarray(l2_bias).reshape(-1)[0]), np.float32)

    xs = [np.asarray(x1), np.asarray(x2)]
    in_maps = []
    for c in range(N_CORES):
        iter_idx = np.empty((N_ITER, 128, IDX_COLS_ITER), np.int16)
        # ovf_idx[p][h]: 1024-idx vector: [tile0 2 blocks][tile1]...[tile3]
        ovf_idx = np.full((2, 2, 1024), ZR, np.int16)
        sovf = np.zeros((N_ITER, 128, 2, OVFB, 128), NP_F8)
        for i in range(N_ITER):
            t, p = i // 2, i % 2
            blk = xs[p][c * B_CORE + t * 128 : c * B_CORE + (t + 1) * 128]
            r_all, f_all = np.nonzero(blk)
            cols = []
            for h in range(2):
                sel = (f_all >= h * HALF) & (f_all < (h + 1) * HALF)
                r, f = r_all[sel], f_all[sel] - h * HALF
                # identity lanes: row r's first B_ID features at blocks 0..,
                # lane r; rest to this (tile, half)'s overflow segment
                blk_idx = np.full((2 * 8, 128), ZR, np.int16)
                pos = np.zeros(128, np.int64)
                ovf_f, ovf_r = [], []
                for rr, ff in zip(r, f):
                    if pos[rr] < B_ID:
                        blk_idx[pos[rr], rr] = ff
                        pos[rr] += 1
                    else:
                        ovf_f.append(ff)
                        ovf_r.append(rr)
                cols.append(_wrap16(blk_idx[:8].reshape(1024)))
                cols.append(_wrap16(blk_idx[8:].reshape(1024)))
                m = len(ovf_f)
                assert m <= OVFB * 128, (c, i, h, m)
                ovf_idx[p, h, OVFB * 128 * t : OVFB * 128 * t + m] = ovf_f
                j = np.arange(m)
                sovf[i, j % 128, h, j // 128, ovf_r] = 1.0
            iter_idx[i] = np.concatenate(cols, axis=1)
        ovf_cols = np.concatenate(
            [_wrap16(ovf_idx[p, h]) for p in range(2) for h in range(2)], axis=1
        )
        idx_full = np.concatenate(
            [ovf_cols, iter_idx.transpose(1, 0, 2).reshape(128, -1)], axis=1
        )
        in_map = dict(tabs)
        in_map.update(
            idx=np.ascontiguousarray(idx_full),
            sovf=np.ascontiguousarray(sovf.reshape(N_ITER, 128, 2 * OVFB * 128)),
            i2=np.ascontiguousarray(i2.reshape(128, 2 * 128)),
            ones=ones,
            b1s=b1s,
            v=v_full,
            b2=b2_full,
        )
        in_maps.append(in_map)
    return in_maps


def _run(x1, x2, l1_weights, l1_biases, l2_weight, l2_bias, trace=False):
    in_maps = _prep(x1, x2, l1_weights, l1_biases, l2_weight, l2_bias)
    if "nc" not in _NC_CACHE:
        _NC_CACHE["nc"] = _build()
    nc = _NC_CACHE["nc"]

    res = run_bass_kernel_spmd(
        nc, in_maps, core_ids=list(range(N_CORES)), trace=trace
    )
    out = np.concatenate(
        [
            np.ascontiguousarray(res.results[c]["out"].T).reshape(B_CORE)
            for c in range(N_CORES)
        ]
    )
    return out.astype(np.float32), res


def kernel(**inputs):
    out, _ = _run(**inputs)
    return out


def kernel_profiled(**inputs):
    _, res = _run(**inputs, trace=True)
    return res


# revision 9
# speedup vs baseline: 1.1359x; 1.0429x over previous
"""NNUE forward kernel for Trainium2, 8-core SPMD, batch-sharded,
sparsity-exploiting (embedding-gather formulation), fp8 tables +
identity-matrix reduction.

Reference computation (B=4096, I=40960, H=256):
    h_p = clip(x_p @ W_p.T + b_p, 0, 1)   for p in {1,2}
    out = concat(h1, h2) @ v + b2         -> (B,)

x_p rows are sparse binary (~30 active features of 40960), so
x_p @ W_p.T is an embedding-sum: h[b] = sum_{active f} W_p.T[f, :].

Tables are fp8 e4m3, pre-scaled by 2^15 so values sit in e4m3's normal
range (weights are ~U(-1/202, 1/202)); the scale folds into the
epilogue: h = clip(psum, 0, SCALE) * (v / SCALE). Measured end-to-end
norm-rel error of e4m3 quantization on this data: 5.9e-3 (tolerance
2e-2).

Identity-matrix reduction: the host assigns each batch row a fixed
budget of B_ID=16 gather lanes per table half: the j-th gathered slot
of a block lands on SBUF partition j%128, and we place row r's
features on lane r. The PE reduction over each pair of 128-slot blocks
is then lhsT = a constant [128,2,128] identity (DoubleRow fp8 matmul,
2x rate), so no per-iteration selector-matrix upload is needed. Rows
with more than 16 features per half spill to a small overflow space
reduced with a tiny uploaded one-hot S. The l1 bias enters the PSUM
group as a rank-1 matmul (ones[1,128] x b1s[1,256]). Unused lanes
gather a zero table row, so no SBUF slot ever holds garbage.

SWDGE gather instructions serialize on the gpsimd engine (~1.3us+
each: 994ns fixed + ring-drain lockstep), so the kernel minimizes
gather COUNT: all overflow slots for all 8 iterations are fetched by 4
upfront 1024-idx gathers (one per (persp, half) table, 2 blocks per
128-row tile), leaving the steady-state loop at exactly 4 1024-idx
gathers per (tile, persp) iteration on the 4 SWDGE queues
(single_packet=False halves SDMA per-descriptor drain cost vs
single-packet mode). Per iteration the PE does 16 identity DoubleRow
matmuls + 2 overflow DoubleRow + 1 bias matmul into PSUM; the Vector
epilogue (relu off PSUM, fused min/dot with v/SCALE, + b2) follows.
No collectives (pure data parallel; batch-sharded).
"""

import numpy as np
import ml_dtypes

import concourse.bass as bass
import concourse.mybir as mybir
from concourse import bacc
from concourse.tile import TileContext
from concourse.bass_utils import run_bass_kernel_spmd

BATCH = 4096
INPUT_SIZE = 40960
HIDDEN = 256
N_CORES = 8
B_CORE = BATCH // N_CORES  # 512
N_TILES = B_CORE // 128  # 4
HALF = INPUT_SIZE // 2  # 20480 rows per table half (int16 index range)
ZR = HALF  # zero row index (padding target)

SCALE = 2.0**15  # fp8 pre-scale; folded into epilogue clip + v
B_ID = 16  # identity-lane budget per row per half
OVFB = 2  # overflow blocks of 128 per (tile, persp, half)
BUFS = 4  # gather pool depth
N_ITER = 2 * N_TILES  # 8 (tile, persp) iterations
IDX_COLS_ITER = 4 * 64  # idx cols per iter (int16 16-wrap)
IDX_COLS_OVF = 4 * 64  # upfront overflow idx cols (4 tables x 1024 idx)

BF16 = mybir.dt.bfloat16
F32 = mybir.dt.float32
F8 = mybir.dt.float8e4
I16 = mybir.dt.int16

NP_F8 = ml_dtypes.float8_e4m3

_NC_CACHE = {}


def _build():
    nc = bacc.Bacc(
        "TRN2",
        target_bir_lowering=False,
        debug=False,
        num_swdge_queues=4,
        dynamic_dma_scratch_size=65536,
    )

    tbl = [
        [
            nc.dram_tensor(f"t{p}{h}", [HALF + 1, HIDDEN], F8, kind="ExternalInput")
            for h in range(2)
        ]
        for p in range(2)
    ]
    idxd = nc.dram_tensor(
        "idx",
        [128, IDX_COLS_OVF + N_ITER * IDX_COLS_ITER],
        I16,
        kind="ExternalInput",
    )
    sovfd = nc.dram_tensor(
        "sovf", [N_ITER, 128, 2 * OVFB * 128], F8, kind="ExternalInput"
    )
    i2d = nc.dram_tensor("i2", [128, 2 * 128], F8, kind="ExternalInput")
    onesd = nc.dram_tensor("ones", [1, 128], F8, kind="ExternalInput")
    b1sd = nc.dram_tensor("b1s", [1, 2 * HIDDEN], F8, kind="ExternalInput")
    vd = nc.dram_tensor("v", [128, 2, HIDDEN], F32, kind="ExternalInput")
    b2d = nc.dram_tensor("b2", [128, 1], F32, kind="ExternalInput")
    outd = nc.dram_tensor("out", [128, N_TILES], F32, kind="ExternalOutput")

    with TileContext(nc) as tc:
        with (
            tc.tile_pool(name="consts", bufs=1) as consts,
            tc.tile_pool(name="gp", bufs=BUFS) as gp,
            tc.tile_pool(name="psum", bufs=6, space="PSUM") as pp,
            tc.tile_pool(name="ep", bufs=2) as ep,
        ):
            # idx goes FIRST on the sync HWDGE queue so the upfront gathers
            # aren't gated on the remaining const uploads (which go out on
            # the scalar HWDGE queue in parallel).
            idxt = consts.tile(
                [128, IDX_COLS_OVF + N_ITER * IDX_COLS_ITER], I16, tag="idx"
            )
            nc.sync.dma_start(out=idxt[:, :], in_=idxd[:, :])
            i2_t = consts.tile([128, 2, 128], F8, tag="i2")
            nc.sync.dma_start(out=i2_t, in_=i2d[:, :])
            ones_t = consts.tile([1, 128], F8, tag="ones")
            nc.sync.dma_start(out=ones_t, in_=onesd[:, :])
            b1s_t = consts.tile([1, 2, HIDDEN], F8, tag="b1s")
            nc.sync.dma_start(out=b1s_t, in_=b1sd[:, :])
            v_t = consts.tile([128, 2, HIDDEN], F32, tag="v")
            nc.scalar.dma_start(out=v_t, in_=vd[:, :, :])
            b2_t = consts.tile([128, 1], F32, tag="b2")
            nc.scalar.dma_start(out=b2_t, in_=b2d[:, :])
            outst = consts.tile([128, N_TILES], F32, tag="outst")
            sovf_t = consts.tile([128, N_ITER, 2, OVFB, 128], F8, tag="sovf")
            for i in range(N_ITER):
                nc.scalar.dma_start(out=sovf_t[:, i, :, :, :], in_=sovfd[i, :, :])

            # Upfront overflow gathers: one 1024-idx gather per (persp,
            # half) table, 2 blocks per tile, fully ZR-padded (no count
            # registers, no garbage).
            ovft = []
            for p in range(2):
                row = []
                for h in range(2):
                    ot = consts.tile(
                        [128, N_TILES * OVFB, HIDDEN], F8, tag=f"ovf{p}{h}"
                    )
                    c0 = (2 * p + h) * 64
                    nc.gpsimd.dma_gather(
                        ot,
                        tbl[p][h][:, :],
                        idxt[:, c0 : c0 + 64],
                        N_TILES * OVFB * 128,
                        N_TILES * OVFB * 128,
                        HIDDEN,
                        single_packet=False,
                        queue_num=2 * p + h,
                    )
                    row.append(ot)
                ovft.append(row)

            acc0 = None
            for i in range(N_ITER):
                t, p = i // 2, i % 2
                ib = IDX_COLS_OVF + i * IDX_COLS_ITER
                # Per half 2 chunks of 1024 (8 blocks each, always full:
                # identity lanes pad with the zero row). Queue map is
                # iteration-stationary.
                gts = []
                for h in range(2):
                    gt = gp.tile([128, 16, HIDDEN], F8, tag=f"g{h}")
                    c0 = ib + 2 * h * 64
                    nc.gpsimd.dma_gather(
                        gt,
                        tbl[p][h][:, :],
                        idxt[:, c0 : c0 + 128],
                        2048,
                        2048,
                        HIDDEN,
                        single_packet=False,
                        queue_num=2 * h + (i % 2),
                    )
                    gts.append(gt)

                psum = pp.tile([128, HIDDEN], F32, tag="psum")
                # l1 bias as a rank-1 matmul: ones[1,128].T @ b1s[1,256].
                nc.tensor.matmul(
                    psum,
                    lhsT=ones_t[0:1, :],
                    rhs=b1s_t[0:1, p, :],
                    start=True,
                    stop=False,
                )
                # Identity DoubleRow matmuls: psum[r,:] += G[r,2c,:]+G[r,2c+1,:]
                for gt in gts:
                    for c2 in range(8):
                        nc.tensor.matmul(
                            psum,
                            lhsT=i2_t[:, :, :],
                            rhs=gt[:, 2 * c2 : 2 * c2 + 2, :],
                            perf_mode=mybir.MatmulPerfMode.DoubleRow,
                            start=False,
                            stop=False,
                        )
                # Overflow: small one-hot S per (iter, half) against the
                # upfront-gathered overflow tile (this tile's 2 blocks).
                for h in range(2):
                    nc.tensor.matmul(
                        psum,
                        lhsT=sovf_t[:, i, h, :, :],
                        rhs=ovft[p][h][:, OVFB * t : OVFB * t + 2, :],
                        perf_mode=mybir.MatmulPerfMode.DoubleRow,
                        start=False,
                        stop=(h == 1),
                    )

                # Epilogue: relu off PSUM, fused (min SCALE, * v/SCALE),
                # reduce; combine perspectives + b2.
                clr = ep.tile([128, HIDDEN], F32, tag="clr")
                nc.vector.tensor_scalar_max(clr, psum, 0.0)
                prod = ep.tile([128, HIDDEN], F32, tag="prod")
                nc.vector.scalar_tensor_tensor(
                    prod,
                    clr,
                    SCALE,
                    v_t[:, p, :],
                    op0=mybir.AluOpType.min,
                    op1=mybir.AluOpType.mult,
                )
                if p == 0:
                    acc0 = ep.tile([128, 1], F32, tag="acc0")
                    nc.vector.tensor_reduce(
                        acc0, prod, axis=mybir.AxisListType.X, op=mybir.AluOpType.add
                    )
                else:
                    acc1 = ep.tile([128, 1], F32, tag="acc1")
                    nc.vector.tensor_reduce(
                        acc1, prod, axis=mybir.AxisListType.X, op=mybir.AluOpType.add
                    )
                    # out[:, t] = (acc0 + b2) + acc1
                    nc.vector.scalar_tensor_tensor(
                        outst[:, t : t + 1],
                        acc0,
                        b2_t,
                        acc1,
                        op0=mybir.AluOpType.add,
                        op1=mybir.AluOpType.add,
                    )
            nc.sync.dma_start(out=outd[:, :], in_=outst)

    nc.compile()
    return nc


def _wrap16(v):
    """Linear idx vector -> [16, n/16] SWDGE wrap, tiled to 128 partitions."""
    n = len(v)
    return np.tile(v.reshape(n // 16, 16).T, (8, 1))  # [128, n//16]


def _prep(x1, x2, l1_weights, l1_biases, l2_weight, l2_bias):
    """Host-side: fp8 tables, per-core identity-lane index lists, overflow
    S matrices, epilogue constants."""
    wt = l1_weights.astype(np.float32).transpose(0, 2, 1)  # [2, I, H]
    tabs = {}
    for p in range(2):
        for h in range(2):
            tt = np.zeros((HALF + 1, HIDDEN), dtype=NP_F8)
            tt[:HALF] = (wt[p, h * HALF : (h + 1) * HALF] * SCALE).astype(NP_F8)
            tabs[f"t{p}{h}"] = tt

    i2 = np.zeros((128, 2, 128), NP_F8)
    for tcol in range(2):
        i2[np.arange(128), tcol, np.arange(128)] = 1.0
    ones = np.ones((1, 128), NP_F8)
    b1s = (l1_biases.astype(np.float32).reshape(1, 2 * HIDDEN) * SCALE).astype(
        NP_F8
    )
    v_full = np.ascontiguousarray(
        np.broadcast_to(
            (l2_weight.astype(np.float32) / SCALE).reshape(1, 2, HIDDEN),
            (128, 2, HIDDEN),
        )
    )
    b2_full = np.full((128, 1), float(np.asarray(l2_bias).reshape(-1)[0]), np.float32)

    xs = [np.asarray(x1), np.asarray(x2)]
    in_maps = []
    for c in range(N_CORES):
        iter_idx = np.empty((N_ITER, 128, IDX_COLS_ITER), np.int16)
        # ovf_idx[p][h]: 1024-idx vector: [tile0 2 blocks][tile1]...[tile3]
        ovf_idx = np.full((2, 2, 1024), ZR, np.int16)
        sovf = np.zeros((N_ITER, 128, 2, OVFB, 128), NP_F8)
        for i in range(N_ITER):
            t, p = i // 2, i % 2
            blk = xs[p][c * B_CORE + t * 128 : c * B_CORE + (t + 1) * 128]
            r_all, f_all = np.nonzero(blk)
            cols = []
            for h in range(2):
                sel = (f_all >= h * HALF) & (f_all < (h + 1) * HALF)
                r, f = r_all[sel], f_all[sel] - h * HALF
                # identity lanes: row r's first B_ID features at blocks 0..,
                # lane r; rest to this (tile, half)'s overflow segment
                blk_idx = np.full((2 * 8, 128), ZR, np.int16)
                pos = np.zeros(128, np.int64)
                ovf_f, ovf_r = [], []
                for rr, ff in zip(r, f):
                    if pos[rr] < B_ID:
                        blk_idx[pos[rr], rr] = ff
                        pos[rr] += 1
                    else:
                        ovf_f.append(ff)
                        ovf_r.append(rr)
                cols.append(_wrap16(blk_idx[:8].reshape(1024)))
                cols.append(_wrap16(blk_idx[8:].reshape(1024)))
                m = len(ovf_f)
                assert m <= OVFB * 128, (c, i, h, m)
                ovf_idx[p, h, OVFB * 128 * t : OVFB * 128 * t + m] = ovf_f
                j = np.arange(m)
                sovf[i, j % 128, h, j // 128, ovf_r] = 1.0
            iter_idx[i] = np.concatenate(cols, axis=1)
        ovf_cols = np.concatenate(
            [_wrap16(ovf_idx[p, h]) for p in range(2) for h in range(2)], axis=1
        )
        idx_full = np.concatenate(
            [ovf_cols, iter_idx.transpose(1, 0, 2).reshape(128, -1)], axis=1
        )
        in_map = dict(tabs)
        in_map.update(
            idx=np.ascontiguousarray(idx_full),
            sovf=np.ascontiguousarray(sovf.reshape(N_ITER, 128, 2 * OVFB * 128)),
            i2=np.ascontiguousarray(i2.reshape(128, 2 * 128)),
            ones=ones,
            b1s=b1s,
            v=v_full,
            b2=b2_full,
        )
        in_maps.append(in_map)
    return in_maps


def _run(x1, x2, l1_weights, l1_biases, l2_weight, l2_bias, trace=False):
    in_maps = _prep(x1, x2, l1_weights, l1_biases, l2_weight, l2_bias)
    if "nc" not in _NC_CACHE:
        _NC_CACHE["nc"] = _build()
    nc = _NC_CACHE["nc"]

    res = run_bass_kernel_spmd(
        nc, in_maps, core_ids=list(range(N_CORES)), trace=trace
    )
    out = np.concatenate(
        [
            np.ascontiguousarray(res.results[c]["out"].T).reshape(B_CORE)
            for c in range(N_CORES)
        ]
    )
    return out.astype(np.float32), res


def kernel(**inputs):
    out, _ = _run(**inputs)
    return out


def kernel_profiled(**inputs):
    _, res = _run(**inputs, trace=True)
    return res
